# revision 39
# baseline (speedup 1.0000x reference)
"""Trainium2 Bass kernel for a 2-layer dense GAT (nn_GAT_87144886436203).

Sharding: row-shard the N=4096 nodes across 8 NeuronCores (512 rows each).
Each core computes attention for its row block against all N columns, with the
contraction axis j on SBUF partitions so `att @ Wh` needs no transposes.

Score factorization: with s = f_i + g_j and alpha = 0.2,
    exp(leaky_relu(s)) = exp(alpha*s) * max(exp((1-alpha)*s), 1)
                       = [exp(alpha*f_i)] * exp(alpha*g_j) * max(G_i * H_j, 1)
where G = exp(0.8 f), H = exp(0.8 g).  The exp(alpha*f_i) factor is constant
per attention row and cancels in the softmax normalization, so it is dropped.
The whole [N, N] exp/leaky-relu work collapses to O(N) vector exps plus, per
[128, 512] score tile, ONE DVE tensor_scalar (4x perf mode)
    C~ = (G_rep * e^{g_j}) max e^{0.2 g_j}   (= e^{0.2 g_j} * max(G H, 1))
and one tensor_tensor mask multiply by the 0/1 adjacency.  The row-sum
(softmax denominator) rides a ones-column through the same matmul.

Engine balance: S_GROUPS build C on ACT as Cm1 = Relu(H_j*G_i - 1) (Prelu
alpha=0, scale=e^{0.8g}, bias=-1) against F'-prescaled weights
Whb~ = e^{0.2 g_j} * [Wh | 1], plus one correction matmul pout += Whb~^T @ adj
per chunk (the "+1").  POOL_COUNT masks can run on GPSIMD.

Wh is computed per-core for OWN rows only and all-gathered ([Wh_h|1]x8 | f,g,
536 cols fp16).  Each layer's gather is split into two half-row "stripes" so
attention on stripe A starts while stripe B is still in flight; groups are
built from the stripe's chunks (two contiguous runs of 2, masks done as two
[128, 1024] tts).  Pair processing is staggered by one group so the ACT unit
and the DVE unit alternate, and epilogues are deferred into the next pair's
first group for overlap.
"""

import numpy as np
import ml_dtypes

import concourse.bass as bass
import concourse.bacc as bacc
import concourse.tile as tile
import concourse.mybir as mybir
from concourse import masks
from concourse.bass_utils import run_bass_kernel_spmd

F16 = mybir.dt.float16
F32 = mybir.dt.float32
NPF16 = ml_dtypes.float16 if hasattr(ml_dtypes, "float16") else np.float16

NCORES = 8
N = 4096            # nodes
K = 512             # input feature dim (= NFEAT)
H = 8               # heads (layer 1)
D = 64              # per-head hidden (= NHID = NCLASS)
DALL = H * D        # 512
R = N // NCORES     # 512 rows per core
JC = N // 128       # 32 j-chunks
G = 4               # chunks per group
NG = JC // G        # 8 groups
AUG1 = D + 1        # 65: [Wh_h | ones]
W1S = H * AUG1      # 520: [Wh_h|1]x8 per-chunk width
CW1 = W1S + 2 * H   # 536: gathered layer-1 row payload [Wh|1]x8 | f,g
CW2 = D + 2         # 66: gathered layer-2 payload [Wh2 | 1 | g2]
ALPHA = 0.2
N_UNITS = H + 1     # 8 heads + layer-2

# Groups are built from stripes: stripe s (0/1) holds chunks {4c+2s, 4c+2s+1}
# (rows [s*256,(s+1)*256) of every core), which arrive with gather stripe s.
# Each group is two contiguous 2-chunk runs.
GROUPS = [
    (8 * k + 2 * s, 8 * k + 2 * s + 1, 8 * k + 2 * s + 4, 8 * k + 2 * s + 5)
    for s in range(2) for k in range(4)
]

# ---- engine-balance knobs ---------------------------------------------- #
S_GROUPS = (1, 3, 5, 7)   # group indices whose C-build runs on ACT
POOL_COUNT = 6            # of the 72 (unit, group) masks, run this many on Pool
NS = len(S_GROUPS)
S_CHUNKS = {}             # chunk -> whbt slot
for _si, _g in enumerate(S_GROUPS):
    for _c, _jc in enumerate(GROUPS[_g]):
        S_CHUNKS[_jc] = _si * G + _c


def _bres(i, count, total):
    return (i * count) // total != ((i + 1) * count) // total


def _mask_on_pool(unit, g):
    if unit == H:       # layer-2 tail is latency-bound; keep masks on DVE
        return False
    return _bres(unit * NG + g, POOL_COUNT, N_UNITS * NG)


_CACHE = {}


# --------------------------------------------------------------------------- #
# device program
# --------------------------------------------------------------------------- #

def _build(emulate_collective=False):
    nc = bacc.Bacc(
        "TRN2",
        target_bir_lowering=False,
        debug=False,
        num_devices=1 if emulate_collective else NCORES,
    )

    xrT = nc.dram_tensor("xrT", [K, R], F16, kind="ExternalInput")
    adjB = nc.dram_tensor("adjB", [N, R], F16, kind="ExternalInput")
    W_all = nc.dram_tensor("W_all", [K, DALL], F16, kind="ExternalInput")
    wa = nc.dram_tensor("wa", [K, 2 * H], F16, kind="ExternalInput")
    W_out = nc.dram_tensor("W_out", [DALL, D], F16, kind="ExternalInput")
    wa2 = nc.dram_tensor("wa2", [DALL, 2], F16, kind="ExternalInput")
    out = nc.dram_tensor("out", [R, D], F32, kind="ExternalOutput")

    with tile.TileContext(nc) as tc:
        _emit(nc, tc, locals(), emulate_collective)

    nc.compile()
    return nc


def _emit(nc, tc, io, emulate_collective):
    xrT, adjB, W_all, wa, W_out, wa2, out = (
        io["xrT"], io["adjB"], io["W_all"], io["wa"],
        io["W_out"], io["wa2"], io["out"],
    )
    AT = mybir.AluOpType
    AF = mybir.ActivationFunctionType

    from contextlib import ExitStack
    with ExitStack() as ctx:
        res = ctx.enter_context(tc.tile_pool(name="res", bufs=1))
        psum = ctx.enter_context(tc.tile_pool(name="psum", bufs=2, space="PSUM"))
        acc = ctx.enter_context(tc.tile_pool(name="acc", bufs=1, space="PSUM"))
        ppool = ctx.enter_context(tc.tile_pool(name="ppool", bufs=4, space="PSUM"))
        work = ctx.enter_context(tc.tile_pool(name="work", bufs=3))
        work2 = ctx.enter_context(tc.tile_pool(name="work2", bufs=3))
        tpool = ctx.enter_context(tc.tile_pool(name="tpool", bufs=4))
        small = ctx.enter_context(tc.tile_pool(name="small", bufs=4))
        rpool = ctx.enter_context(tc.tile_pool(name="rpool", bufs=2))
        dram = ctx.enter_context(tc.tile_pool(name="dram", bufs=1, space="DRAM"))

        # ---- resident SBUF tensors ---- #
        xrT_sb = res.tile([128, 4 * R], F16, tag="xrT")
        adjB_sb = res.tile([128, JC * R], F16, tag="adjB")
        W_all_sb = res.tile([128, 4 * DALL], F16, tag="W_all")
        wa_sb = res.tile([128, 4 * 2 * H], F16, tag="wa")
        W_out_sb = res.tile([128, 4 * D], F16, tag="W_out")
        wa2_sb = res.tile([128, 4 * 2], F16, tag="wa2")
        whbig_sb = res.tile([128, JC * CW1], F16, tag="whbig")  # gathered L1
        whbt_sb = res.tile([128, max(NS, 1) * G * W1S], F16, tag="whbt")  # F'-scaled
        eg_sb = res.tile([128, JC * H], F32, tag="eg")     # exp(g)
        e02_sb = res.tile([128, JC * H], F32, tag="e02")   # exp(0.2 g)
        e08_sb = res.tile([128, JC * H], F32, tag="e08")   # exp(0.8 g)
        e02h_sb = res.tile([128, JC * H], F16, tag="e02h")
        hcatT_sb = res.tile([128, 4 * R], F16, tag="hcatT")
        whb2_sb = res.tile([128, JC * CW2], F16, tag="whb2")
        whb2t_sb = res.tile([128, max(NS, 1) * G * AUG1], F16, tag="whb2t")
        eg2_sb = res.tile([128, JC], F32, tag="eg2")
        e022_sb = res.tile([128, JC], F32, tag="e022")
        e082_sb = res.tile([128, JC], F32, tag="e082")
        ones_sb = res.tile([1, 128], F32, tag="ones")
        ones16_sb = res.tile([1, 128], F16, tag="ones16")
        neg1_sb = res.tile([128, 1], F32, tag="neg1")
        ident_sb = res.tile([64, 64], F32, tag="ident")
        out_sb = res.tile([128, 4 * D], F32, tag="out_sb")

        def chunked(dram_t, width):
            return dram_t.ap().rearrange("(c p) w -> p c w", p=128)

        def chunked_sb(sb_ap, width):
            return sb_ap.rearrange("p (c w) -> p c w", w=width)

        def load(sb_tile, dram_t, width, split=1):
            dst = chunked_sb(sb_tile[:], width)
            src = chunked(dram_t, width)
            nch = dst.shape[1]
            step = max(1, nch // split)
            for lo in range(0, nch, step):
                hi = min(nch, lo + step)
                nc.sync.dma_start(dst[:, lo:hi, :], src[:, lo:hi, :])

        whbig_ch = chunked_sb(whbig_sb[:], CW1)
        whb2_ch = chunked_sb(whb2_sb[:], CW2)

        def pe_warm(n):
            # keep the PE p-state ramped through DMA-wait windows
            for _ in range(n):
                scr = psum.tile([128, R], F32, tag="bank", name="warm")
                nc.tensor.matmul(scr[:], xrT_sb[:, 0:128], xrT_sb[:, 0:R],
                                 start=True, stop=True)

        # ---- phase 0: weight loads + constants ---- #
        xrT_dst = chunked_sb(xrT_sb[:], R)
        xrT_src = chunked(xrT, R)
        nc.sync.dma_start(xrT_dst[:, :, 0:256], xrT_src[:, :, 0:256])
        load(wa_sb, wa, 2 * H)
        load(W_all_sb, W_all, DALL)
        nc.sync.dma_start(xrT_dst[:, :, 256:R], xrT_src[:, :, 256:R])
        load(W_out_sb, W_out, D)
        load(wa2_sb, wa2, 2)
        nc.vector.memset(ones_sb[:], 1.0)
        nc.vector.memset(ones16_sb[:], 1.0)
        nc.vector.memset(neg1_sb[:], -1.0)
        masks.make_identity(nc, ident_sb[:])

        # ---- phase A: own-row Wh/f/g + G-row, striped allgather ---- #
        pfr_t = ppool.tile([16, R], F32, tag="pout", name="pfr")
        pfr = pfr_t[:]
        gx16 = res.tile([16, R], F16, tag="gx16")
        fgb_d = dram.tile([16, R], F16, tag="fgb")

        def pfr_emit():
            for kc in range(4):
                nc.tensor.matmul(
                    pfr, wa_sb[:, kc * 2 * H:(kc + 1) * 2 * H],
                    xrT_sb[:, kc * R:(kc + 1) * R],
                    start=(kc == 0), stop=(kc == 3),
                )
            nc.scalar.activation(gx16[:], pfr, AF.Exp, scale=1.0 - ALPHA)
            nc.gpsimd.dma_start(fgb_d[:], gx16[:])

        gt1s = [res.tile([128, 2 * CW1], F16, tag=f"gt1{s}", name=f"gt1{s}")
                for s in range(2)]
        for s in range(2):
            nc.gpsimd.memset(gt1s[s][:], 1.0)   # bakes the ones columns

        def own_block(ib):
            gt1 = gt1s[ib // 2][:, (ib % 2) * CW1:(ib % 2 + 1) * CW1]
            pw = psum.tile([128, DALL], F32, tag="bank")
            pf = psum.tile([128, 2 * H], F32, tag="bank")
            for kc in range(4):
                lhsT = xrT_sb[:, kc * R + ib * 128: kc * R + (ib + 1) * 128]
                nc.tensor.matmul(
                    pw[:], lhsT, W_all_sb[:, kc * DALL:(kc + 1) * DALL],
                    start=(kc == 0), stop=(kc == 3))
                nc.tensor.matmul(
                    pf[:], lhsT, wa_sb[:, kc * 2 * H:(kc + 1) * 2 * H],
                    start=(kc == 0), stop=(kc == 3))
            dst = gt1[:, 0:W1S].rearrange(
                "p (h x) -> p h x", x=AUG1)[:, :, 0:D]
            nc.scalar.activation(
                dst, pw.rearrange("p (h x) -> p h x", x=D), AF.Copy)
            nc.vector.tensor_copy(gt1[:, W1S:CW1], pf[:])

        cc_space = {} if emulate_collective else {"addr_space": "Shared"}
        RH = R // 2
        cc1_in = [dram.tile([RH, CW1], F16, tag=f"cc1_in{s}", name=f"cc1_in{s}") for s in range(2)]
        cc1_out = [dram.tile([NCORES * RH, CW1], F16, tag=f"cc1_out{s}",
                             name=f"cc1_out{s}", **cc_space) for s in range(2)]

        def gather_in(cc_in, src_sb_ch):
            nc.sync.dma_start(
                cc_in[:].rearrange("(c p) w -> p c w", p=128),
                src_sb_ch)

        def gather_piece(cc_in, cc_out, nchunks, c2):
            """Deliver cores [c2, c2+2)'s rows of the allgather output."""
            if emulate_collective:
                w = cc_in.shape[1]
                dst = cc_out[:].rearrange("(c q) w -> c q w", c=NCORES)
                nc.sync.dma_start(
                    dst[c2:c2 + 2],
                    cc_in[:].unsqueeze(0).broadcast_to(
                        [2, nchunks * 128, w]))
            elif c2 == 0:
                nc.gpsimd.collective_compute(
                    "AllGather", mybir.AluOpType.bypass,
                    replica_groups=[list(range(NCORES))],
                    ins=[cc_in.opt()], outs=[cc_out.opt()],
                )

        def gather(cc_in, cc_out, src_sb_ch, nchunks):
            gather_in(cc_in, src_sb_ch)
            for c2 in range(0, NCORES, 2):
                gather_piece(cc_in, cc_out, nchunks, c2)

        def land_stripe(s, cc_out, ch_ap, q0=0, q1=NCORES):
            # DMA APs are limited to 3 dims: one DMA per chunk-of-pair t
            src = cc_out[:].rearrange("(co t p) w -> p co t w", t=2, p=128)
            dst = ch_ap.rearrange("p (co fo) w -> p co fo w", fo=4)
            for t in range(2):
                nc.sync.dma_start(
                    dst[:, q0:q1, 2 * s + t, :],
                    src[:, q0:q1, t, :])

        def adj_stripe(s, q0=0, q1=NCORES, piece=2):
            src = adjB.ap().rearrange("(co fo p) w -> p co fo w", fo=4, p=128)
            dst = chunked_sb(adjB_sb[:], R).rearrange(
                "p (co fo) w -> p co fo w", fo=4)
            for t in range(2):
                for q in range(q0, q1, piece):
                    nc.sync.dma_start(
                        dst[:, q:q + piece, 2 * s + t, :],
                        src[:, q:q + piece, 2 * s + t, :])

        def l1_etiles(s, q0=0, q1=NCORES, step=NCORES):
            wview = whbig_sb[:].rearrange("p (co fo w) -> p co fo w", fo=4, w=CW1)
            step = min(step, q1 - q0)
            for q in range(q0, q1, step):
                for t in range(2):
                    gc = wview[:, q:q + step, 2 * s + t, W1S:CW1].rearrange(
                        "p co (h two) -> p co h two", two=2)[:, :, :, 1:2]
                    for e_sb, sc in ((eg_sb, 1.0), (e02_sb, ALPHA),
                                     (e08_sb, 1.0 - ALPHA)):
                        dst = e_sb[:].rearrange(
                            "p (co fo h) -> p co fo h", fo=4, h=H)[
                            :, q:q + step, 2 * s + t, :].unsqueeze(3)
                        nc.scalar.activation(dst, gc, AF.Exp, scale=sc)
                    esrc = e02_sb[:].rearrange(
                        "p (co fo h) -> p co fo h", fo=4, h=H)[
                        :, q:q + step, 2 * s + t, :]
                    edst = e02h_sb[:].rearrange(
                        "p (co fo h) -> p co fo h", fo=4, h=H)[
                        :, q:q + step, 2 * s + t, :]
                    nc.vector.tensor_copy(edst, esrc)

        def l1_whbt(stripe, only_g=None):
            for g in S_GROUPS:
                if (g // 4) != stripe or (only_g is not None and g != only_g):
                    continue
                for jc in GROUPS[g]:
                    k = S_CHUNKS[jc]
                    src = whbig_ch[:, jc, 0:W1S].rearrange(
                        "p (h x) -> p h x", x=AUG1)
                    fb = e02h_sb[:, jc * H:(jc + 1) * H].unsqueeze(2) \
                        .broadcast_to([128, H, AUG1])
                    nc.gpsimd.tensor_tensor(
                        whbt_sb[:, k * W1S:(k + 1) * W1S].rearrange(
                            "p (h x) -> p h x", x=AUG1),
                        src, fb, AT.mult)

        # emission order = DMA queue order: stripe A lands in co-pair
        # pieces (gather -> land -> exps -> whbt), each unlocking one group,
        # before stripe B so compute starts as early as possible.
        own_block(0)
        own_block(1)
        pe_warm(12)
        adj_stripe(0)
        gather_in(cc1_in[0], chunked_sb(gt1s[0][:], CW1))
        for c2 in range(4):
            gather_piece(cc1_in[0], cc1_out[0], 2, 2 * c2)
            land_stripe(0, cc1_out[0], whbig_ch, q0=2 * c2, q1=2 * c2 + 2)
            l1_etiles(0, q0=2 * c2, q1=2 * c2 + 2)
            if c2 in S_GROUPS:
                l1_whbt(0, only_g=c2)
        own_block(2)
        own_block(3)
        pfr_emit()
        gather_in(cc1_in[1], chunked_sb(gt1s[1][:], CW1))
        for c2 in range(4):
            gather_piece(cc1_in[1], cc1_out[1], 2, 2 * c2)
            land_stripe(1, cc1_out[1], whbig_ch, q0=2 * c2, q1=2 * c2 + 2)
            adj_stripe(1, q0=2 * c2, q1=2 * c2 + 2)
        l1_etiles(1)
        l1_whbt(1)

        # ---- attention unit ---- #
        def unit_start(f_row_dram):
            pout = ppool.tile([AUG1, R], F32, tag="pout")
            frep = tpool.tile([128, R], F16, tag="frep")
            nc.sync.dma_start(frep[:], f_row_dram.broadcast_to([128, R]))
            return pout, frep

        def unit_group(unit, pout, frep, g, mm, lhsT_of, lhsTs_of,
                       eg_of, e02_of, e08_of):
            on_act = g in S_GROUPS
            chunks = GROUPS[g]
            u = work.tile([128, G * R], F16, tag="u")
            for c, jc in enumerate(chunks):
                if on_act:
                    nc.scalar.activation(
                        u[:, c * R:(c + 1) * R], frep[:],
                        AF.Prelu, bias=neg1_sb[:], scale=e08_of(jc),
                        alpha=0.0)
                else:
                    nc.vector.tensor_scalar(
                        u[:, c * R:(c + 1) * R], frep[:],
                        eg_of(jc), e02_of(jc), AT.mult, AT.max)
            pm = work2.tile([128, G * R], F16, tag="pm")
            eng = nc.gpsimd if _mask_on_pool(unit, g) else nc.vector
            for r in range(2):
                c0 = chunks[2 * r]
                eng.tensor_tensor(
                    pm[:, 2 * r * R:(2 * r + 2) * R],
                    u[:, 2 * r * R:(2 * r + 2) * R],
                    adjB_sb[:, c0 * R:(c0 + 2) * R], AT.mult)
            for c, jc in enumerate(chunks):
                lhsT = lhsTs_of(S_CHUNKS[jc]) if on_act else lhsT_of(jc)
                nc.tensor.matmul(
                    pout[:], lhsT, pm[:, c * R:(c + 1) * R],
                    start=(mm[0] == 0), stop=(mm[0] == mm[1] - 1))
                mm[0] += 1
            if on_act:
                for jc in chunks:
                    nc.tensor.matmul(
                        pout[:], lhsTs_of(S_CHUNKS[jc]),
                        adjB_sb[:, jc * R:(jc + 1) * R],
                        start=(mm[0] == 0), stop=(mm[0] == mm[1] - 1))
                    mm[0] += 1

        def epilogue(pout, dst_ap, dst_f32):
            """dst = elu(att_out / rowsum) written to dst_ap ([64, R])."""
            dt = F32 if dst_f32 else F16
            recip = rpool.tile([1, R], F32, tag="recip")
            nc.vector.reciprocal(recip[:], pout[D:D + 1, :])
            pr = psum.tile([D, R], F32, tag="bank")
            nc.tensor.matmul(pr[:], ones_sb[0:1, 0:D], recip[:])
            rsb = small.tile([D, R], F32, tag="ep")
            nc.scalar.activation(rsb[:], pr[:], AF.Copy)
            hl = small.tile([D, R], dt, tag="ep")
            nc.vector.tensor_tensor(hl[:], pout[0:D, :], rsb[:], AT.mult)
            # elu(x) = max(x,0) + min(exp(x),1) - 1   (exp monotone)
            q = small.tile([D, R], dt, tag="ep")
            nc.scalar.activation(q[:], hl[:], AF.Exp)
            t1 = small.tile([D, R], dt, tag="ep")
            nc.vector.tensor_scalar(t1[:], q[:], 1.0, -1.0, AT.min, AT.add)
            t2 = small.tile([D, R], dt, tag="ep")
            nc.gpsimd.tensor_scalar(t2[:], hl[:], 0.0, None, AT.max)
            nc.gpsimd.tensor_tensor(dst_ap, t1[:], t2[:], AT.add)

        # ---- phase C: layer-1 heads, pairs w/ deferred epilogues ---- #
        # Layer-2 prep partials accumulate in SBUF (a PSUM accumulator held
        # open across the whole layer-1 phase corrupts on real HW).
        MM_TOTAL = JC + NS * G
        pw2acc = res.tile([128, 4 * CW2], F32, tag="pw2acc")
        pfg2 = res.tile([2, R], F32, tag="pfg2")

        def l2_accum(kc):
            pt2 = psum.tile([128, 4 * CW2], F32, tag="bank", name="pt2")
            for ib in range(4):
                lhsT = hcatT_sb[:, kc * R + ib * 128: kc * R + (ib + 1) * 128]
                nc.tensor.matmul(
                    pt2[:, ib * CW2: ib * CW2 + D],
                    lhsT, W_out_sb[:, kc * D:(kc + 1) * D],
                    start=True, stop=True)
                nc.tensor.matmul(
                    pt2[:, ib * CW2 + D: ib * CW2 + D + 2],
                    lhsT, wa2_sb[:, kc * 2:(kc + 1) * 2],
                    start=True, stop=True)
            ptf = psum.tile([2, R], F32, tag="bank", name="ptf")
            nc.tensor.matmul(ptf[:], wa2_sb[:, kc * 2:(kc + 1) * 2],
                             hcatT_sb[:, kc * R:(kc + 1) * R],
                             start=True, stop=True)
            if kc == 0:
                nc.vector.tensor_copy(pw2acc[:], pt2[:])
                nc.vector.tensor_copy(pfg2[:], ptf[:])
            else:
                nc.vector.tensor_tensor(pw2acc[:], pw2acc[:], pt2[:], AT.add)
                nc.vector.tensor_tensor(pfg2[:], pfg2[:], ptf[:], AT.add)

        def l1_args(h):
            return (
                lambda jc, h=h: whbig_ch[:, jc, h * AUG1:(h + 1) * AUG1],
                lambda k, h=h: whbt_sb[:, k * W1S + h * AUG1:
                                       k * W1S + (h + 1) * AUG1],
                lambda jc, h=h: eg_sb[:, jc * H + h: jc * H + h + 1],
                lambda jc, h=h: e02_sb[:, jc * H + h: jc * H + h + 1],
                lambda jc, h=h: e08_sb[:, jc * H + h: jc * H + h + 1],
            )

        prev_pair = None
        started = {}

        def ensure_started(hp):
            if hp not in started and hp < H:
                started[hp] = [
                    [h] + list(unit_start(fgb_d[2 * h:2 * h + 1, :]))
                    + [[0, MM_TOTAL], l1_args(h)]
                    for h in (hp, hp + 1)
                ]
            return started.get(hp)

        for hp in range(0, H, 2):
            pair = ensure_started(hp)
            for gi in range(NG):
                for pi, (h, pout, frep, mm, args) in enumerate(pair):
                    # stagger the pair by one group so one unit is in an
                    # ACT-built group while the other is in a DVE-built one
                    unit_group(h, pout, frep, (gi + pi) % NG, mm, *args)
                # spread the previous pair's epilogues and the layer-2
                # partial accumulation across three group slots so their
                # cross-engine chains don't block the in-order queues
                if prev_pair is not None and gi == 0:
                    for (h, pout, frep, mm, args) in prev_pair:
                        kc, po = h // 2, (h % 2) * D
                        epilogue(pout,
                                 hcatT_sb[po:po + D, kc * R:(kc + 1) * R],
                                 dst_f32=False)
                if prev_pair is not None and gi == 1:
                    l2_accum(prev_pair[0][0] // 2)
                if gi == NG - 2:
                    ensure_started(hp + 2)
            prev_pair = pair
        for (h, pout, frep, mm, args) in prev_pair:
            kc, po = h // 2, (h % 2) * D
            epilogue(pout, hcatT_sb[po:po + D, kc * R:(kc + 1) * R],
                     dst_f32=False)
        l2_accum(3)

        # ---- phase D: layer-2 gather (striped) ---- #
        gt2s = [res.tile([128, 2 * CW2], F16, tag=f"gt2{s}", name=f"gt2{s}")
                for s in range(2)]
        for s in range(2):
            nc.vector.memset(gt2s[s][:], 1.0)
        for ib in range(4):
            gt2 = gt2s[ib // 2][:, (ib % 2) * CW2:(ib % 2 + 1) * CW2]
            nc.vector.tensor_copy(
                gt2[:, 0:D], pw2acc[:, ib * CW2: ib * CW2 + D])
            nc.vector.tensor_copy(
                gt2[:, D + 1:D + 2],
                pw2acc[:, ib * CW2 + D + 1: ib * CW2 + D + 2])

        # broadcast G2row across partitions via PE instead of a DRAM bounce
        g2row = res.tile([1, R], F16, tag="g2row")
        nc.scalar.activation(g2row[:], pfg2[0:1, :], AF.Exp, scale=1.0 - ALPHA)
        frep2 = tpool.tile([128, R], F16, tag="frep")
        pb2 = psum.tile([128, R], F32, tag="bank")
        nc.tensor.matmul(pb2[:], ones16_sb[0:1, 0:128], g2row[:],
                         start=True, stop=True)
        nc.vector.tensor_copy(frep2[:], pb2[:])

        cc2_in = [dram.tile([RH, CW2], F16, tag=f"cc2_in{s}", name=f"cc2_in{s}") for s in range(2)]
        cc2_out = [dram.tile([NCORES * RH, CW2], F16, tag=f"cc2_out{s}",
                             name=f"cc2_out{s}", **cc_space) for s in range(2)]

        def l2_etiles(s):
            wview = whb2_sb[:].rearrange("p (co fo w) -> p co fo w", fo=4, w=CW2)
            for t in range(2):
                gc = wview[:, :, 2 * s + t, D + 1:D + 2]
                for e_sb, sc in ((eg2_sb, 1.0), (e022_sb, ALPHA),
                                 (e082_sb, 1.0 - ALPHA)):
                    dst = e_sb[:].rearrange(
                        "p (co fo) -> p co fo", fo=4)[:, :, 2 * s + t] \
                        .unsqueeze(2)
                    nc.scalar.activation(dst, gc, AF.Exp, scale=sc)

        def l2_whbt(stripe):
            for g in S_GROUPS:
                if (g // 4) != stripe:
                    continue
                for jc in GROUPS[g]:
                    k = S_CHUNKS[jc]
                    nc.vector.tensor_scalar(
                        whb2t_sb[:, k * AUG1:(k + 1) * AUG1],
                        whb2_ch[:, jc, 0:AUG1],
                        e022_sb[:, jc:jc + 1], None, AT.mult)

        # ---- phase E: layer 2, stripe-pipelined ---- #
        pout2 = ppool.tile([AUG1, R], F32, tag="pout")
        mm2 = [0, MM_TOTAL]
        args2 = (
            lambda jc: whb2_ch[:, jc, 0:AUG1],
            lambda k: whb2t_sb[:, k * AUG1:(k + 1) * AUG1],
            lambda jc: eg2_sb[:, jc:jc + 1],
            lambda jc: e022_sb[:, jc:jc + 1],
            lambda jc: e082_sb[:, jc:jc + 1],
        )
        for s in range(2):
            gather(cc2_in[s], cc2_out[s], chunked_sb(gt2s[s][:], CW2), 2)
            land_stripe(s, cc2_out[s], whb2_ch)
            l2_etiles(s)
            l2_whbt(s)

        def pe_warm2(n):
            # gt2-gated fillers: run during the layer-2 gather wait
            for _ in range(n):
                scr = psum.tile([128, 2 * CW2], F32, tag="bank", name="warm2")
                nc.tensor.matmul(scr[:], gt2s[0][:, 0:128], gt2s[0][:],
                                 start=True, stop=True)

        pe_warm2(30)
        for g in range(NG):
            unit_group(H, pout2, frep2, g, mm2, *args2)
        # final epilogue, block-pipelined with the transposes and out DMA
        recip2 = rpool.tile([1, R], F32, tag="recip")
        nc.vector.reciprocal(recip2[:], pout2[D:D + 1, :])
        pr2 = psum.tile([D, R], F32, tag="bank")
        nc.tensor.matmul(pr2[:], ones_sb[0:1, 0:D], recip2[:])
        out_ch = out.ap().rearrange("(c p) w -> p c w", p=128)
        for ib in range(4):
            cs = slice(ib * 128, (ib + 1) * 128)
            rsb = small.tile([D, 128], F32, tag="ep")
            nc.scalar.activation(rsb[:], pr2[:, cs], AF.Copy)
            hl = small.tile([D, 128], F32, tag="ep")
            nc.vector.tensor_tensor(hl[:], pout2[0:D, cs], rsb[:], AT.mult)
            q = small.tile([D, 128], F32, tag="ep")
            nc.scalar.activation(q[:], hl[:], AF.Exp)
            t1 = small.tile([D, 128], F32, tag="ep")
            nc.vector.tensor_scalar(t1[:], q[:], 1.0, -1.0, AT.min, AT.add)
            t2 = small.tile([D, 128], F32, tag="ep")
            nc.gpsimd.tensor_scalar(t2[:], hl[:], 0.0, None, AT.max)
            r2 = small.tile([D, 128], F32, tag="ep2")
            nc.gpsimd.tensor_tensor(r2[:], t1[:], t2[:], AT.add)
            pt = psum.tile([128, D], F32, tag="bank")
            nc.tensor.transpose(pt[:], r2[:], ident_sb[:])
            nc.vector.tensor_copy(out_sb[:, ib * D:(ib + 1) * D], pt[:])
            nc.sync.dma_start(out_ch[:, ib, :],
                              chunked_sb(out_sb[:], D)[:, ib, :])


# --------------------------------------------------------------------------- #
# host side
# --------------------------------------------------------------------------- #

def _pack_inputs(x, adj, W_heads, a_src, a_dst, W_out, a_src_out, a_dst_out):
    """Shard + repack the full inputs into the 8 per-core input maps."""
    x = np.asarray(x, np.float32)
    adj = np.asarray(adj)
    W_heads = np.asarray(W_heads, np.float32)
    a_src = np.asarray(a_src, np.float32)
    a_dst = np.asarray(a_dst, np.float32)
    W_out_np = np.asarray(W_out, np.float32)
    a_src_out = np.asarray(a_src_out, np.float32)
    a_dst_out = np.asarray(a_dst_out, np.float32)

    f16 = NPF16
    W_all = np.ascontiguousarray(
        W_heads.transpose(1, 0, 2).reshape(K, DALL)).astype(f16)     # [K, H*D]
    wa_cols = []
    for h in range(H):
        wa_cols.append(W_heads[h] @ a_src[h])
        wa_cols.append(W_heads[h] @ a_dst[h])
    wa = np.stack(wa_cols, axis=1).astype(f16)                       # [K, 16]
    W_out_p = W_out_np.astype(f16)                                   # [DALL, D]
    wa2 = np.stack([W_out_np @ a_src_out, W_out_np @ a_dst_out],
                   axis=1).astype(f16)                               # [DALL, 2]

    in_maps = []
    for c in range(NCORES):
        rows = slice(c * R, (c + 1) * R)
        adj_rows = (adj[rows, :] > 0).astype(np.float32)             # [R, N]
        adjB = np.ascontiguousarray(adj_rows.T).astype(f16)          # [N, R] 0/1
        in_maps.append({
            "xrT": np.ascontiguousarray(x[rows].T).astype(f16),
            "adjB": adjB,
            "W_all": W_all,
            "wa": wa,
            "W_out": W_out_p,
            "wa2": wa2,
        })
    return in_maps


def kernel(**inputs) -> np.ndarray:
    if "nc" not in _CACHE:
        _CACHE["nc"] = _build(emulate_collective=False)
    nc = _CACHE["nc"]
    in_maps = _pack_inputs(**inputs)
    res = run_bass_kernel_spmd(nc, in_maps, core_ids=list(range(NCORES)))
    return np.concatenate([res.results[c]["out"] for c in range(NCORES)], axis=0)


# revision 40
# speedup vs baseline: 1.0619x; 1.0619x over previous
"""Trainium2 Bass kernel for a 2-layer dense GAT (nn_GAT_87144886436203).

Sharding: row-shard the N=4096 nodes across 8 NeuronCores (512 rows each).
Each core computes attention for its row block against all N columns, with the
contraction axis j on SBUF partitions so `att @ Wh` needs no transposes.

Score factorization: with s = f_i + g_j and alpha = 0.2,
    exp(leaky_relu(s)) = exp(alpha*s) * max(exp((1-alpha)*s), 1)
                       = [exp(alpha*f_i)] * exp(alpha*g_j) * max(G_i * H_j, 1)
where G = exp(0.8 f), H = exp(0.8 g).  The exp(alpha*f_i) factor is constant
per attention row and cancels in the softmax normalization, so it is dropped.
The whole [N, N] exp/leaky-relu work collapses to O(N) vector exps plus, per
[128, 512] score tile, ONE DVE tensor_scalar (4x perf mode)
    C~ = (G_rep * e^{g_j}) max e^{0.2 g_j}   (= e^{0.2 g_j} * max(G H, 1))
and one tensor_tensor mask multiply by the 0/1 adjacency.  The row-sum
(softmax denominator) rides a ones-column through the same matmul.

Engine balance: S_GROUPS build C on ACT as Cm1 = Relu(H_j*G_i - 1) (Prelu
alpha=0, scale=e^{0.8g}, bias=-1) against F'-prescaled weights
Whb~ = e^{0.2 g_j} * [Wh | 1], plus one correction matmul pout += Whb~^T @ adj
per chunk (the "+1").  POOL_COUNT masks can run on GPSIMD.

Wh is computed per-core for OWN rows only and all-gathered ([Wh_h|1]x8 | f,g,
536 cols fp16).  Each layer's gather is split into two half-row "stripes" so
attention on stripe A starts while stripe B is still in flight; groups are
built from the stripe's chunks (two contiguous runs of 2, masks done as two
[128, 1024] tts).  Pair processing is staggered by one group so the ACT unit
and the DVE unit alternate, and epilogues are deferred into the next pair's
first group for overlap.
"""

import numpy as np
import ml_dtypes

import concourse.bass as bass
import concourse.bacc as bacc
import concourse.tile as tile
import concourse.mybir as mybir
from concourse import masks
from concourse.bass_utils import run_bass_kernel_spmd

F16 = mybir.dt.float16
F32 = mybir.dt.float32
NPF16 = ml_dtypes.float16 if hasattr(ml_dtypes, "float16") else np.float16

NCORES = 8
N = 4096            # nodes
K = 512             # input feature dim (= NFEAT)
H = 8               # heads (layer 1)
D = 64              # per-head hidden (= NHID = NCLASS)
DALL = H * D        # 512
R = N // NCORES     # 512 rows per core
JC = N // 128       # 32 j-chunks
G = 4               # chunks per group
NG = JC // G        # 8 groups
AUG1 = D + 1        # 65: [Wh_h | ones]
W1S = H * AUG1      # 520: [Wh_h|1]x8 per-chunk width
CW1 = W1S + 2 * H   # 536: gathered layer-1 row payload [Wh|1]x8 | f,g
CW2 = D + 2         # 66: gathered layer-2 payload [Wh2 | 1 | g2]
ALPHA = 0.2
N_UNITS = H + 1     # 8 heads + layer-2

# Groups are built from stripes: stripe s (0/1) holds chunks {4c+2s, 4c+2s+1}
# (rows [s*256,(s+1)*256) of every core), which arrive with gather stripe s.
# Each group is two contiguous 2-chunk runs.
GROUPS = [
    (8 * k + 2 * s, 8 * k + 2 * s + 1, 8 * k + 2 * s + 4, 8 * k + 2 * s + 5)
    for s in range(2) for k in range(4)
]

# ---- engine-balance knobs ---------------------------------------------- #
S_GROUPS = (1, 3, 5, 7)   # group indices whose C-build runs on ACT
POOL_COUNT = 6            # of the 72 (unit, group) masks, run this many on Pool
NS = len(S_GROUPS)
S_CHUNKS = {}             # chunk -> whbt slot
for _si, _g in enumerate(S_GROUPS):
    for _c, _jc in enumerate(GROUPS[_g]):
        S_CHUNKS[_jc] = _si * G + _c


def _bres(i, count, total):
    return (i * count) // total != ((i + 1) * count) // total


def _mask_on_pool(unit, g):
    if unit == H:       # layer-2 tail is latency-bound; keep masks on DVE
        return False
    return _bres(unit * NG + g, POOL_COUNT, N_UNITS * NG)


_CACHE = {}


# --------------------------------------------------------------------------- #
# device program
# --------------------------------------------------------------------------- #

def _build(emulate_collective=False):
    nc = bacc.Bacc(
        "TRN2",
        target_bir_lowering=False,
        debug=False,
        num_devices=1 if emulate_collective else NCORES,
    )

    xrT = nc.dram_tensor("xrT", [K, R], F16, kind="ExternalInput")
    adjB = nc.dram_tensor("adjB", [N, R], F16, kind="ExternalInput")
    W_all = nc.dram_tensor("W_all", [K, DALL], F16, kind="ExternalInput")
    wa = nc.dram_tensor("wa", [K, 2 * H], F16, kind="ExternalInput")
    W_out = nc.dram_tensor("W_out", [DALL, D], F16, kind="ExternalInput")
    wa2 = nc.dram_tensor("wa2", [DALL, 2], F16, kind="ExternalInput")
    out = nc.dram_tensor("out", [R, D], F32, kind="ExternalOutput")

    with tile.TileContext(nc) as tc:
        _emit(nc, tc, locals(), emulate_collective)

    nc.compile()
    return nc


def _emit(nc, tc, io, emulate_collective):
    xrT, adjB, W_all, wa, W_out, wa2, out = (
        io["xrT"], io["adjB"], io["W_all"], io["wa"],
        io["W_out"], io["wa2"], io["out"],
    )
    AT = mybir.AluOpType
    AF = mybir.ActivationFunctionType

    from contextlib import ExitStack
    with ExitStack() as ctx:
        res = ctx.enter_context(tc.tile_pool(name="res", bufs=1))
        psum = ctx.enter_context(tc.tile_pool(name="psum", bufs=2, space="PSUM"))
        acc = ctx.enter_context(tc.tile_pool(name="acc", bufs=1, space="PSUM"))
        ppool = ctx.enter_context(tc.tile_pool(name="ppool", bufs=4, space="PSUM"))
        work = ctx.enter_context(tc.tile_pool(name="work", bufs=3))
        work2 = ctx.enter_context(tc.tile_pool(name="work2", bufs=3))
        tpool = ctx.enter_context(tc.tile_pool(name="tpool", bufs=4))
        small = ctx.enter_context(tc.tile_pool(name="small", bufs=4))
        rpool = ctx.enter_context(tc.tile_pool(name="rpool", bufs=2))
        dram = ctx.enter_context(tc.tile_pool(name="dram", bufs=1, space="DRAM"))

        # ---- resident SBUF tensors ---- #
        xrT_sb = res.tile([128, 4 * R], F16, tag="xrT")
        adjB_sb = res.tile([128, JC * R], F16, tag="adjB")
        W_all_sb = res.tile([128, 4 * DALL], F16, tag="W_all")
        wa_sb = res.tile([128, 4 * 2 * H], F16, tag="wa")
        W_out_sb = res.tile([128, 4 * D], F16, tag="W_out")
        wa2_sb = res.tile([128, 4 * 2], F16, tag="wa2")
        whbig_sb = res.tile([128, JC * CW1], F16, tag="whbig")  # gathered L1
        whbt_sb = res.tile([128, max(NS, 1) * G * W1S], F16, tag="whbt")  # F'-scaled
        eg_sb = res.tile([128, JC * H], F32, tag="eg")     # exp(g)
        e02_sb = res.tile([128, JC * H], F32, tag="e02")   # exp(0.2 g)
        e08_sb = res.tile([128, JC * H], F32, tag="e08")   # exp(0.8 g)
        e02h_sb = res.tile([128, JC * H], F16, tag="e02h")
        hcatT_sb = res.tile([128, 4 * R], F16, tag="hcatT")
        whb2_sb = res.tile([128, JC * CW2], F16, tag="whb2")
        whb2t_sb = res.tile([128, max(NS, 1) * G * AUG1], F16, tag="whb2t")
        eg2_sb = res.tile([128, JC], F32, tag="eg2")
        e022_sb = res.tile([128, JC], F32, tag="e022")
        e082_sb = res.tile([128, JC], F32, tag="e082")
        ones_sb = res.tile([1, 128], F32, tag="ones")
        ones16_sb = res.tile([1, 128], F16, tag="ones16")
        neg1_sb = res.tile([128, 1], F32, tag="neg1")
        ident_sb = res.tile([64, 64], F32, tag="ident")
        out_sb = res.tile([128, 4 * D], F32, tag="out_sb")

        def chunked(dram_t, width):
            return dram_t.ap().rearrange("(c p) w -> p c w", p=128)

        def chunked_sb(sb_ap, width):
            return sb_ap.rearrange("p (c w) -> p c w", w=width)

        def load(sb_tile, dram_t, width, split=1):
            dst = chunked_sb(sb_tile[:], width)
            src = chunked(dram_t, width)
            nch = dst.shape[1]
            step = max(1, nch // split)
            for lo in range(0, nch, step):
                hi = min(nch, lo + step)
                nc.sync.dma_start(dst[:, lo:hi, :], src[:, lo:hi, :])

        whbig_ch = chunked_sb(whbig_sb[:], CW1)
        whb2_ch = chunked_sb(whb2_sb[:], CW2)

        def pe_warm(n):
            # keep the PE p-state ramped through DMA-wait windows
            for _ in range(n):
                scr = psum.tile([128, R], F32, tag="bank", name="warm")
                nc.tensor.matmul(scr[:], xrT_sb[:, 0:128], xrT_sb[:, 0:R],
                                 start=True, stop=True)

        # ---- phase 0: weight loads + constants ---- #
        xrT_dst = chunked_sb(xrT_sb[:], R)
        xrT_src = chunked(xrT, R)
        nc.sync.dma_start(xrT_dst[:, :, 0:256], xrT_src[:, :, 0:256])
        load(wa_sb, wa, 2 * H)
        load(W_all_sb, W_all, DALL)
        nc.sync.dma_start(xrT_dst[:, :, 256:R], xrT_src[:, :, 256:R])
        load(W_out_sb, W_out, D)
        load(wa2_sb, wa2, 2)
        nc.vector.memset(ones_sb[:], 1.0)
        nc.vector.memset(ones16_sb[:], 1.0)
        nc.vector.memset(neg1_sb[:], -1.0)
        masks.make_identity(nc, ident_sb[:])

        # ---- phase A: own-row Wh/f/g + G-row, striped allgather ---- #
        pfr_t = ppool.tile([16, R], F32, tag="pout", name="pfr")
        pfr = pfr_t[:]
        gx16 = res.tile([16, R], F16, tag="gx16")
        fgb_d = dram.tile([16, R], F16, tag="fgb")

        def pfr_emit():
            for kc in range(4):
                nc.tensor.matmul(
                    pfr, wa_sb[:, kc * 2 * H:(kc + 1) * 2 * H],
                    xrT_sb[:, kc * R:(kc + 1) * R],
                    start=(kc == 0), stop=(kc == 3),
                )
            nc.scalar.activation(gx16[:], pfr, AF.Exp, scale=1.0 - ALPHA)
            nc.gpsimd.dma_start(fgb_d[:], gx16[:])

        gt1s = [res.tile([128, 2 * CW1], F16, tag=f"gt1{s}", name=f"gt1{s}")
                for s in range(2)]
        for s in range(2):
            nc.gpsimd.memset(gt1s[s][:], 1.0)   # bakes the ones columns

        def own_block(ib):
            gt1 = gt1s[ib // 2][:, (ib % 2) * CW1:(ib % 2 + 1) * CW1]
            pw = psum.tile([128, DALL], F32, tag="bank")
            pf = psum.tile([128, 2 * H], F32, tag="bank")
            for kc in range(4):
                lhsT = xrT_sb[:, kc * R + ib * 128: kc * R + (ib + 1) * 128]
                nc.tensor.matmul(
                    pw[:], lhsT, W_all_sb[:, kc * DALL:(kc + 1) * DALL],
                    start=(kc == 0), stop=(kc == 3))
                nc.tensor.matmul(
                    pf[:], lhsT, wa_sb[:, kc * 2 * H:(kc + 1) * 2 * H],
                    start=(kc == 0), stop=(kc == 3))
            dst = gt1[:, 0:W1S].rearrange(
                "p (h x) -> p h x", x=AUG1)[:, :, 0:D]
            nc.scalar.activation(
                dst, pw.rearrange("p (h x) -> p h x", x=D), AF.Copy)
            nc.vector.tensor_copy(gt1[:, W1S:CW1], pf[:])

        cc_space = {} if emulate_collective else {"addr_space": "Shared"}
        RH = R // 2
        cc1_in = [dram.tile([RH, CW1], F16, tag=f"cc1_in{s}", name=f"cc1_in{s}") for s in range(2)]
        cc1_out = [dram.tile([NCORES * RH, CW1], F16, tag=f"cc1_out{s}",
                             name=f"cc1_out{s}", **cc_space) for s in range(2)]

        def gather_in(cc_in, src_sb_ch):
            nc.sync.dma_start(
                cc_in[:].rearrange("(c p) w -> p c w", p=128),
                src_sb_ch)

        def gather_piece(cc_in, cc_out, nchunks, c2):
            """Deliver cores [c2, c2+2)'s rows of the allgather output."""
            if emulate_collective:
                w = cc_in.shape[1]
                dst = cc_out[:].rearrange("(c q) w -> c q w", c=NCORES)
                nc.sync.dma_start(
                    dst[c2:c2 + 2],
                    cc_in[:].unsqueeze(0).broadcast_to(
                        [2, nchunks * 128, w]))
            elif c2 == 0:
                nc.gpsimd.collective_compute(
                    "AllGather", mybir.AluOpType.bypass,
                    replica_groups=[list(range(NCORES))],
                    ins=[cc_in.opt()], outs=[cc_out.opt()],
                )

        def gather(cc_in, cc_out, src_sb_ch, nchunks):
            gather_in(cc_in, src_sb_ch)
            for c2 in range(0, NCORES, 2):
                gather_piece(cc_in, cc_out, nchunks, c2)

        def land_stripe(s, cc_out, ch_ap, q0=0, q1=NCORES):
            # DMA APs are limited to 3 dims: one DMA per chunk-of-pair t
            src = cc_out[:].rearrange("(co t p) w -> p co t w", t=2, p=128)
            dst = ch_ap.rearrange("p (co fo) w -> p co fo w", fo=4)
            for t in range(2):
                nc.sync.dma_start(
                    dst[:, q0:q1, 2 * s + t, :],
                    src[:, q0:q1, t, :])

        def adj_stripe(s, q0=0, q1=NCORES, piece=2):
            src = adjB.ap().rearrange("(co fo p) w -> p co fo w", fo=4, p=128)
            dst = chunked_sb(adjB_sb[:], R).rearrange(
                "p (co fo) w -> p co fo w", fo=4)
            for t in range(2):
                for q in range(q0, q1, piece):
                    nc.sync.dma_start(
                        dst[:, q:q + piece, 2 * s + t, :],
                        src[:, q:q + piece, 2 * s + t, :])

        def l1_etiles(s, q0=0, q1=NCORES, step=NCORES):
            wview = whbig_sb[:].rearrange("p (co fo w) -> p co fo w", fo=4, w=CW1)
            step = min(step, q1 - q0)
            for q in range(q0, q1, step):
                for t in range(2):
                    gc = wview[:, q:q + step, 2 * s + t, W1S:CW1].rearrange(
                        "p co (h two) -> p co h two", two=2)[:, :, :, 1:2]
                    for e_sb, sc in ((eg_sb, 1.0), (e02_sb, ALPHA),
                                     (e08_sb, 1.0 - ALPHA)):
                        dst = e_sb[:].rearrange(
                            "p (co fo h) -> p co fo h", fo=4, h=H)[
                            :, q:q + step, 2 * s + t, :].unsqueeze(3)
                        nc.scalar.activation(dst, gc, AF.Exp, scale=sc)
                    esrc = e02_sb[:].rearrange(
                        "p (co fo h) -> p co fo h", fo=4, h=H)[
                        :, q:q + step, 2 * s + t, :]
                    edst = e02h_sb[:].rearrange(
                        "p (co fo h) -> p co fo h", fo=4, h=H)[
                        :, q:q + step, 2 * s + t, :]
                    nc.vector.tensor_copy(edst, esrc)

        def l1_whbt(stripe, only_g=None):
            for g in S_GROUPS:
                if (g // 4) != stripe or (only_g is not None and g != only_g):
                    continue
                for jc in GROUPS[g]:
                    k = S_CHUNKS[jc]
                    src = whbig_ch[:, jc, 0:W1S].rearrange(
                        "p (h x) -> p h x", x=AUG1)
                    fb = e02h_sb[:, jc * H:(jc + 1) * H].unsqueeze(2) \
                        .broadcast_to([128, H, AUG1])
                    nc.gpsimd.tensor_tensor(
                        whbt_sb[:, k * W1S:(k + 1) * W1S].rearrange(
                            "p (h x) -> p h x", x=AUG1),
                        src, fb, AT.mult)

        # emission order = DMA queue order: stripe A lands in co-pair
        # pieces (gather -> land -> exps -> whbt), each unlocking one group,
        # before stripe B so compute starts as early as possible.
        own_block(0)
        own_block(1)
        pe_warm(12)
        adj_stripe(0)
        gather_in(cc1_in[0], chunked_sb(gt1s[0][:], CW1))
        for c2 in range(4):
            gather_piece(cc1_in[0], cc1_out[0], 2, 2 * c2)
            land_stripe(0, cc1_out[0], whbig_ch, q0=2 * c2, q1=2 * c2 + 2)
            l1_etiles(0, q0=2 * c2, q1=2 * c2 + 2)
            if c2 in S_GROUPS:
                l1_whbt(0, only_g=c2)
        own_block(2)
        own_block(3)
        pfr_emit()
        gather(cc1_in[1], cc1_out[1], chunked_sb(gt1s[1][:], CW1), 2)
        land_stripe(1, cc1_out[1], whbig_ch)
        l1_etiles(1)
        l1_whbt(1)

        # ---- attention unit ---- #
        def unit_start(f_row_dram):
            pout = ppool.tile([AUG1, R], F32, tag="pout")
            frep = tpool.tile([128, R], F16, tag="frep")
            nc.sync.dma_start(frep[:], f_row_dram.broadcast_to([128, R]))
            return pout, frep

        def unit_group(unit, pout, frep, g, mm, lhsT_of, lhsTs_of,
                       eg_of, e02_of, e08_of):
            on_act = g in S_GROUPS
            chunks = GROUPS[g]
            u = work.tile([128, G * R], F16, tag="u")
            for c, jc in enumerate(chunks):
                if on_act:
                    nc.scalar.activation(
                        u[:, c * R:(c + 1) * R], frep[:],
                        AF.Prelu, bias=neg1_sb[:], scale=e08_of(jc),
                        alpha=0.0)
                else:
                    nc.vector.tensor_scalar(
                        u[:, c * R:(c + 1) * R], frep[:],
                        eg_of(jc), e02_of(jc), AT.mult, AT.max)
            pm = work2.tile([128, G * R], F16, tag="pm")
            eng = nc.gpsimd if _mask_on_pool(unit, g) else nc.vector
            for r in range(2):
                c0 = chunks[2 * r]
                eng.tensor_tensor(
                    pm[:, 2 * r * R:(2 * r + 2) * R],
                    u[:, 2 * r * R:(2 * r + 2) * R],
                    adjB_sb[:, c0 * R:(c0 + 2) * R], AT.mult)
            for c, jc in enumerate(chunks):
                lhsT = lhsTs_of(S_CHUNKS[jc]) if on_act else lhsT_of(jc)
                nc.tensor.matmul(
                    pout[:], lhsT, pm[:, c * R:(c + 1) * R],
                    start=(mm[0] == 0), stop=(mm[0] == mm[1] - 1))
                mm[0] += 1
            if on_act:
                for jc in chunks:
                    nc.tensor.matmul(
                        pout[:], lhsTs_of(S_CHUNKS[jc]),
                        adjB_sb[:, jc * R:(jc + 1) * R],
                        start=(mm[0] == 0), stop=(mm[0] == mm[1] - 1))
                    mm[0] += 1

        def epilogue(pout, dst_ap, dst_f32):
            """dst = elu(att_out / rowsum) written to dst_ap ([64, R])."""
            dt = F32 if dst_f32 else F16
            recip = rpool.tile([1, R], F32, tag="recip")
            nc.vector.reciprocal(recip[:], pout[D:D + 1, :])
            pr = psum.tile([D, R], F32, tag="bank")
            nc.tensor.matmul(pr[:], ones_sb[0:1, 0:D], recip[:])
            rsb = small.tile([D, R], F32, tag="ep")
            nc.scalar.activation(rsb[:], pr[:], AF.Copy)
            hl = small.tile([D, R], dt, tag="ep")
            nc.vector.tensor_tensor(hl[:], pout[0:D, :], rsb[:], AT.mult)
            # elu(x) = max(x,0) + min(exp(x),1) - 1   (exp monotone)
            q = small.tile([D, R], dt, tag="ep")
            nc.scalar.activation(q[:], hl[:], AF.Exp)
            t1 = small.tile([D, R], dt, tag="ep")
            nc.vector.tensor_scalar(t1[:], q[:], 1.0, -1.0, AT.min, AT.add)
            t2 = small.tile([D, R], dt, tag="ep")
            nc.gpsimd.tensor_scalar(t2[:], hl[:], 0.0, None, AT.max)
            nc.gpsimd.tensor_tensor(dst_ap, t1[:], t2[:], AT.add)

        # ---- phase C: layer-1 heads, pairs w/ deferred epilogues ---- #
        # Layer-2 prep partials accumulate in SBUF (a PSUM accumulator held
        # open across the whole layer-1 phase corrupts on real HW).
        MM_TOTAL = JC + NS * G
        pw2acc = res.tile([128, 4 * CW2], F32, tag="pw2acc")
        pfg2 = res.tile([2, R], F32, tag="pfg2")

        def l2_accum(kc):
            pt2 = psum.tile([128, 4 * CW2], F32, tag="bank", name="pt2")
            for ib in range(4):
                lhsT = hcatT_sb[:, kc * R + ib * 128: kc * R + (ib + 1) * 128]
                nc.tensor.matmul(
                    pt2[:, ib * CW2: ib * CW2 + D],
                    lhsT, W_out_sb[:, kc * D:(kc + 1) * D],
                    start=True, stop=True)
                nc.tensor.matmul(
                    pt2[:, ib * CW2 + D: ib * CW2 + D + 2],
                    lhsT, wa2_sb[:, kc * 2:(kc + 1) * 2],
                    start=True, stop=True)
            ptf = psum.tile([2, R], F32, tag="bank", name="ptf")
            nc.tensor.matmul(ptf[:], wa2_sb[:, kc * 2:(kc + 1) * 2],
                             hcatT_sb[:, kc * R:(kc + 1) * R],
                             start=True, stop=True)
            if kc == 0:
                nc.vector.tensor_copy(pw2acc[:], pt2[:])
                nc.vector.tensor_copy(pfg2[:], ptf[:])
            else:
                nc.vector.tensor_tensor(pw2acc[:], pw2acc[:], pt2[:], AT.add)
                nc.vector.tensor_tensor(pfg2[:], pfg2[:], ptf[:], AT.add)

        def l1_args(h):
            return (
                lambda jc, h=h: whbig_ch[:, jc, h * AUG1:(h + 1) * AUG1],
                lambda k, h=h: whbt_sb[:, k * W1S + h * AUG1:
                                       k * W1S + (h + 1) * AUG1],
                lambda jc, h=h: eg_sb[:, jc * H + h: jc * H + h + 1],
                lambda jc, h=h: e02_sb[:, jc * H + h: jc * H + h + 1],
                lambda jc, h=h: e08_sb[:, jc * H + h: jc * H + h + 1],
            )

        prev_pair = None
        started = {}

        def ensure_started(hp):
            if hp not in started and hp < H:
                started[hp] = [
                    [h] + list(unit_start(fgb_d[2 * h:2 * h + 1, :]))
                    + [[0, MM_TOTAL], l1_args(h)]
                    for h in (hp, hp + 1)
                ]
            return started.get(hp)

        for hp in range(0, H, 2):
            pair = ensure_started(hp)
            for gi in range(NG):
                for pi, (h, pout, frep, mm, args) in enumerate(pair):
                    # stagger the pair by one group so one unit is in an
                    # ACT-built group while the other is in a DVE-built one
                    unit_group(h, pout, frep, (gi + pi) % NG, mm, *args)
                if gi == 0 and prev_pair is None and hp == 0:
                    adj_stripe(1)
                # spread the previous pair's epilogues and the layer-2
                # partial accumulation across three group slots so their
                # cross-engine chains don't block the in-order queues
                if prev_pair is not None and gi == 0:
                    for (h, pout, frep, mm, args) in prev_pair:
                        kc, po = h // 2, (h % 2) * D
                        epilogue(pout,
                                 hcatT_sb[po:po + D, kc * R:(kc + 1) * R],
                                 dst_f32=False)
                if prev_pair is not None and gi == 1:
                    l2_accum(prev_pair[0][0] // 2)
                if gi == NG - 2:
                    ensure_started(hp + 2)
            prev_pair = pair
        for (h, pout, frep, mm, args) in prev_pair:
            kc, po = h // 2, (h % 2) * D
            epilogue(pout, hcatT_sb[po:po + D, kc * R:(kc + 1) * R],
                     dst_f32=False)
        l2_accum(3)

        # ---- phase D: layer-2 gather (striped) ---- #
        gt2s = [res.tile([128, 2 * CW2], F16, tag=f"gt2{s}", name=f"gt2{s}")
                for s in range(2)]
        for s in range(2):
            nc.vector.memset(gt2s[s][:], 1.0)
        for ib in range(4):
            gt2 = gt2s[ib // 2][:, (ib % 2) * CW2:(ib % 2 + 1) * CW2]
            nc.vector.tensor_copy(
                gt2[:, 0:D], pw2acc[:, ib * CW2: ib * CW2 + D])
            nc.vector.tensor_copy(
                gt2[:, D + 1:D + 2],
                pw2acc[:, ib * CW2 + D + 1: ib * CW2 + D + 2])

        # broadcast G2row across partitions via PE instead of a DRAM bounce
        g2row = res.tile([1, R], F16, tag="g2row")
        nc.scalar.activation(g2row[:], pfg2[0:1, :], AF.Exp, scale=1.0 - ALPHA)
        frep2 = tpool.tile([128, R], F16, tag="frep")
        pb2 = psum.tile([128, R], F32, tag="bank")
        nc.tensor.matmul(pb2[:], ones16_sb[0:1, 0:128], g2row[:],
                         start=True, stop=True)
        nc.vector.tensor_copy(frep2[:], pb2[:])

        cc2_in = [dram.tile([RH, CW2], F16, tag=f"cc2_in{s}", name=f"cc2_in{s}") for s in range(2)]
        cc2_out = [dram.tile([NCORES * RH, CW2], F16, tag=f"cc2_out{s}",
                             name=f"cc2_out{s}", **cc_space) for s in range(2)]

        def l2_etiles(s):
            wview = whb2_sb[:].rearrange("p (co fo w) -> p co fo w", fo=4, w=CW2)
            for t in range(2):
                gc = wview[:, :, 2 * s + t, D + 1:D + 2]
                for e_sb, sc in ((eg2_sb, 1.0), (e022_sb, ALPHA),
                                 (e082_sb, 1.0 - ALPHA)):
                    dst = e_sb[:].rearrange(
                        "p (co fo) -> p co fo", fo=4)[:, :, 2 * s + t] \
                        .unsqueeze(2)
                    nc.scalar.activation(dst, gc, AF.Exp, scale=sc)

        def l2_whbt(stripe):
            for g in S_GROUPS:
                if (g // 4) != stripe:
                    continue
                for jc in GROUPS[g]:
                    k = S_CHUNKS[jc]
                    nc.vector.tensor_scalar(
                        whb2t_sb[:, k * AUG1:(k + 1) * AUG1],
                        whb2_ch[:, jc, 0:AUG1],
                        e022_sb[:, jc:jc + 1], None, AT.mult)

        # ---- phase E: layer 2, stripe-pipelined ---- #
        pout2 = ppool.tile([AUG1, R], F32, tag="pout")
        mm2 = [0, MM_TOTAL]
        args2 = (
            lambda jc: whb2_ch[:, jc, 0:AUG1],
            lambda k: whb2t_sb[:, k * AUG1:(k + 1) * AUG1],
            lambda jc: eg2_sb[:, jc:jc + 1],
            lambda jc: e022_sb[:, jc:jc + 1],
            lambda jc: e082_sb[:, jc:jc + 1],
        )
        for s in range(2):
            gather(cc2_in[s], cc2_out[s], chunked_sb(gt2s[s][:], CW2), 2)
            land_stripe(s, cc2_out[s], whb2_ch)
            l2_etiles(s)
            l2_whbt(s)

        def pe_warm2(n):
            # gt2-gated fillers: run during the layer-2 gather wait
            for _ in range(n):
                scr = psum.tile([128, 2 * CW2], F32, tag="bank", name="warm2")
                nc.tensor.matmul(scr[:], gt2s[0][:, 0:128], gt2s[0][:],
                                 start=True, stop=True)

        pe_warm2(30)
        for g in range(NG):
            unit_group(H, pout2, frep2, g, mm2, *args2)
        # final epilogue, block-pipelined with the transposes and out DMA
        recip2 = rpool.tile([1, R], F32, tag="recip")
        nc.vector.reciprocal(recip2[:], pout2[D:D + 1, :])
        pr2 = psum.tile([D, R], F32, tag="bank")
        nc.tensor.matmul(pr2[:], ones_sb[0:1, 0:D], recip2[:])
        out_ch = out.ap().rearrange("(c p) w -> p c w", p=128)
        for ib in range(4):
            cs = slice(ib * 128, (ib + 1) * 128)
            rsb = small.tile([D, 128], F32, tag="ep")
            nc.scalar.activation(rsb[:], pr2[:, cs], AF.Copy)
            hl = small.tile([D, 128], F32, tag="ep")
            nc.vector.tensor_tensor(hl[:], pout2[0:D, cs], rsb[:], AT.mult)
            q = small.tile([D, 128], F32, tag="ep")
            nc.scalar.activation(q[:], hl[:], AF.Exp)
            t1 = small.tile([D, 128], F32, tag="ep")
            nc.vector.tensor_scalar(t1[:], q[:], 1.0, -1.0, AT.min, AT.add)
            t2 = small.tile([D, 128], F32, tag="ep")
            nc.gpsimd.tensor_scalar(t2[:], hl[:], 0.0, None, AT.max)
            r2 = small.tile([D, 128], F32, tag="ep2")
            nc.gpsimd.tensor_tensor(r2[:], t1[:], t2[:], AT.add)
            pt = psum.tile([128, D], F32, tag="bank")
            nc.tensor.transpose(pt[:], r2[:], ident_sb[:])
            nc.vector.tensor_copy(out_sb[:, ib * D:(ib + 1) * D], pt[:])
            nc.sync.dma_start(out_ch[:, ib, :],
                              chunked_sb(out_sb[:], D)[:, ib, :])


# --------------------------------------------------------------------------- #
# host side
# --------------------------------------------------------------------------- #

def _pack_inputs(x, adj, W_heads, a_src, a_dst, W_out, a_src_out, a_dst_out):
    """Shard + repack the full inputs into the 8 per-core input maps."""
    x = np.asarray(x, np.float32)
    adj = np.asarray(adj)
    W_heads = np.asarray(W_heads, np.float32)
    a_src = np.asarray(a_src, np.float32)
    a_dst = np.asarray(a_dst, np.float32)
    W_out_np = np.asarray(W_out, np.float32)
    a_src_out = np.asarray(a_src_out, np.float32)
    a_dst_out = np.asarray(a_dst_out, np.float32)

    f16 = NPF16
    W_all = np.ascontiguousarray(
        W_heads.transpose(1, 0, 2).reshape(K, DALL)).astype(f16)     # [K, H*D]
    wa_cols = []
    for h in range(H):
        wa_cols.append(W_heads[h] @ a_src[h])
        wa_cols.append(W_heads[h] @ a_dst[h])
    wa = np.stack(wa_cols, axis=1).astype(f16)                       # [K, 16]
    W_out_p = W_out_np.astype(f16)                                   # [DALL, D]
    wa2 = np.stack([W_out_np @ a_src_out, W_out_np @ a_dst_out],
                   axis=1).astype(f16)                               # [DALL, 2]

    in_maps = []
    for c in range(NCORES):
        rows = slice(c * R, (c + 1) * R)
        adj_rows = (adj[rows, :] > 0).astype(np.float32)             # [R, N]
        adjB = np.ascontiguousarray(adj_rows.T).astype(f16)          # [N, R] 0/1
        in_maps.append({
            "xrT": np.ascontiguousarray(x[rows].T).astype(f16),
            "adjB": adjB,
            "W_all": W_all,
            "wa": wa,
            "W_out": W_out_p,
            "wa2": wa2,
        })
    return in_maps


def kernel(**inputs) -> np.ndarray:
    if "nc" not in _CACHE:
        _CACHE["nc"] = _build(emulate_collective=False)
    nc = _CACHE["nc"]
    in_maps = _pack_inputs(**inputs)
    res = run_bass_kernel_spmd(nc, in_maps, core_ids=list(range(NCORES)))
    return np.concatenate([res.results[c]["out"] for c in range(NCORES)], axis=0)


# revision 45
# speedup vs baseline: 1.0775x; 1.0147x over previous
"""Trainium2 Bass kernel for a 2-layer dense GAT (nn_GAT_87144886436203).

Sharding: row-shard the N=4096 nodes across 8 NeuronCores (512 rows each).
Each core computes attention for its row block against all N columns, with the
contraction axis j on SBUF partitions so `att @ Wh` needs no transposes.

Score factorization: with s = f_i + g_j and alpha = 0.2,
    exp(leaky_relu(s)) = exp(alpha*s) * max(exp((1-alpha)*s), 1)
                       = [exp(alpha*f_i)] * exp(alpha*g_j) * max(G_i * H_j, 1)
where G = exp(0.8 f), H = exp(0.8 g).  The exp(alpha*f_i) factor is constant
per attention row and cancels in the softmax normalization, so it is dropped.
The whole [N, N] exp/leaky-relu work collapses to O(N) vector exps plus, per
[128, 512] score tile, ONE DVE tensor_scalar (4x perf mode)
    C~ = (G_rep * e^{g_j}) max e^{0.2 g_j}   (= e^{0.2 g_j} * max(G H, 1))
and one tensor_tensor mask multiply by the 0/1 adjacency.  The row-sum
(softmax denominator) rides a ones-column through the same matmul.

Engine balance: S_GROUPS build C on ACT as Cm1 = Relu(H_j*G_i - 1) (Prelu
alpha=0, scale=e^{0.8g}, bias=-1) against F'-prescaled weights
Whb~ = e^{0.2 g_j} * [Wh | 1], plus one correction matmul pout += Whb~^T @ adj
per chunk (the "+1").  POOL_COUNT masks can run on GPSIMD.

Wh is computed per-core for OWN rows only and all-gathered ([Wh_h|1]x8 | f,g,
536 cols fp16).  Each layer's gather is split into two half-row "stripes" so
attention on stripe A starts while stripe B is still in flight; groups are
built from the stripe's chunks (two contiguous runs of 2, masks done as two
[128, 1024] tts).  Pair processing is staggered by one group so the ACT unit
and the DVE unit alternate, and epilogues are deferred into the next pair's
first group for overlap.
"""

import numpy as np
import ml_dtypes

import concourse.bass as bass
import concourse.bacc as bacc
import concourse.tile as tile
import concourse.mybir as mybir
from concourse import masks
from concourse.bass_utils import run_bass_kernel_spmd

F16 = mybir.dt.float16
F32 = mybir.dt.float32
NPF16 = ml_dtypes.float16 if hasattr(ml_dtypes, "float16") else np.float16

NCORES = 8
N = 4096            # nodes
K = 512             # input feature dim (= NFEAT)
H = 8               # heads (layer 1)
D = 64              # per-head hidden (= NHID = NCLASS)
DALL = H * D        # 512
R = N // NCORES     # 512 rows per core
JC = N // 128       # 32 j-chunks
G = 4               # chunks per group
NG = JC // G        # 8 groups
AUG1 = D + 1        # 65: [Wh_h | ones]
W1S = H * AUG1      # 520: [Wh_h|1]x8 per-chunk width
CW1 = W1S + 2 * H   # 536: gathered layer-1 row payload [Wh|1]x8 | f,g
CW2 = D + 2         # 66: gathered layer-2 payload [Wh2 | 1 | g2]
ALPHA = 0.2
N_UNITS = H + 1     # 8 heads + layer-2

# Groups are built from stripes: stripe s (0/1) holds chunks {4c+2s, 4c+2s+1}
# (rows [s*256,(s+1)*256) of every core), which arrive with gather stripe s.
# Each group is two contiguous 2-chunk runs.
GROUPS = [
    (8 * k + 2 * s, 8 * k + 2 * s + 1, 8 * k + 2 * s + 4, 8 * k + 2 * s + 5)
    for s in range(2) for k in range(4)
]

# ---- engine-balance knobs ---------------------------------------------- #
S_GROUPS = (1, 3, 5, 7)   # group indices whose C-build runs on ACT
POOL_COUNT = 6            # of the 72 (unit, group) masks, run this many on Pool
NS = len(S_GROUPS)
S_CHUNKS = {}             # chunk -> whbt slot
for _si, _g in enumerate(S_GROUPS):
    for _c, _jc in enumerate(GROUPS[_g]):
        S_CHUNKS[_jc] = _si * G + _c


def _bres(i, count, total):
    return (i * count) // total != ((i + 1) * count) // total


def _mask_on_pool(unit, g):
    if unit == H:       # layer-2 tail is latency-bound; keep masks on DVE
        return False
    return _bres(unit * NG + g, POOL_COUNT, N_UNITS * NG)


_CACHE = {}


# --------------------------------------------------------------------------- #
# device program
# --------------------------------------------------------------------------- #

def _build(emulate_collective=False):
    nc = bacc.Bacc(
        "TRN2",
        target_bir_lowering=False,
        debug=False,
        num_devices=1 if emulate_collective else NCORES,
    )

    xrT = nc.dram_tensor("xrT", [K, R], F16, kind="ExternalInput")
    adjB = nc.dram_tensor("adjB", [N, R], F16, kind="ExternalInput")
    W_all = nc.dram_tensor("W_all", [K, DALL], F16, kind="ExternalInput")
    wa = nc.dram_tensor("wa", [K, 2 * H], F16, kind="ExternalInput")
    W_out = nc.dram_tensor("W_out", [DALL, D], F16, kind="ExternalInput")
    wa2 = nc.dram_tensor("wa2", [DALL, 2], F16, kind="ExternalInput")
    out = nc.dram_tensor("out", [R, D], F32, kind="ExternalOutput")

    with tile.TileContext(nc) as tc:
        _emit(nc, tc, locals(), emulate_collective)

    nc.compile()
    return nc


def _emit(nc, tc, io, emulate_collective):
    xrT, adjB, W_all, wa, W_out, wa2, out = (
        io["xrT"], io["adjB"], io["W_all"], io["wa"],
        io["W_out"], io["wa2"], io["out"],
    )
    AT = mybir.AluOpType
    AF = mybir.ActivationFunctionType

    from contextlib import ExitStack
    with ExitStack() as ctx:
        res = ctx.enter_context(tc.tile_pool(name="res", bufs=1))
        psum = ctx.enter_context(tc.tile_pool(name="psum", bufs=2, space="PSUM"))
        acc = ctx.enter_context(tc.tile_pool(name="acc", bufs=1, space="PSUM"))
        ppool = ctx.enter_context(tc.tile_pool(name="ppool", bufs=4, space="PSUM"))
        work = ctx.enter_context(tc.tile_pool(name="work", bufs=3))
        work2 = ctx.enter_context(tc.tile_pool(name="work2", bufs=3))
        tpool = ctx.enter_context(tc.tile_pool(name="tpool", bufs=4))
        small = ctx.enter_context(tc.tile_pool(name="small", bufs=4))
        rpool = ctx.enter_context(tc.tile_pool(name="rpool", bufs=2))
        dram = ctx.enter_context(tc.tile_pool(name="dram", bufs=1, space="DRAM"))

        # ---- resident SBUF tensors ---- #
        xrT_sb = res.tile([128, 4 * R], F16, tag="xrT")
        adjB_sb = res.tile([128, JC * R], F16, tag="adjB")
        W_all_sb = res.tile([128, 4 * DALL], F16, tag="W_all")
        wa_sb = res.tile([128, 4 * 2 * H], F16, tag="wa")
        W_out_sb = res.tile([128, 4 * D], F16, tag="W_out")
        wa2_sb = res.tile([128, 4 * 2], F16, tag="wa2")
        whbig_sb = res.tile([128, JC * CW1], F16, tag="whbig")  # gathered L1
        whbt_sb = res.tile([128, max(NS, 1) * G * W1S], F16, tag="whbt")  # F'-scaled
        eg_sb = res.tile([128, JC * H], F32, tag="eg")     # exp(g)
        e02_sb = res.tile([128, JC * H], F32, tag="e02")   # exp(0.2 g)
        e08_sb = res.tile([128, JC * H], F32, tag="e08")   # exp(0.8 g)
        e02h_sb = res.tile([128, JC * H], F16, tag="e02h")
        hcatT_sb = res.tile([128, 4 * R], F16, tag="hcatT")
        whb2_sb = res.tile([128, JC * CW2], F16, tag="whb2")
        whb2t_sb = res.tile([128, max(NS, 1) * G * AUG1], F16, tag="whb2t")
        eg2_sb = res.tile([128, JC], F32, tag="eg2")
        e022_sb = res.tile([128, JC], F32, tag="e022")
        e082_sb = res.tile([128, JC], F32, tag="e082")
        ones_sb = res.tile([1, 128], F32, tag="ones")
        ones16_sb = res.tile([1, 128], F16, tag="ones16")
        neg1_sb = res.tile([128, 1], F32, tag="neg1")
        ident_sb = res.tile([64, 64], F32, tag="ident")
        out_sb = res.tile([128, 4 * D], F32, tag="out_sb")

        def chunked(dram_t, width):
            return dram_t.ap().rearrange("(c p) w -> p c w", p=128)

        def chunked_sb(sb_ap, width):
            return sb_ap.rearrange("p (c w) -> p c w", w=width)

        def load(sb_tile, dram_t, width, split=1):
            dst = chunked_sb(sb_tile[:], width)
            src = chunked(dram_t, width)
            nch = dst.shape[1]
            step = max(1, nch // split)
            for lo in range(0, nch, step):
                hi = min(nch, lo + step)
                nc.sync.dma_start(dst[:, lo:hi, :], src[:, lo:hi, :])

        whbig_ch = chunked_sb(whbig_sb[:], CW1)
        whb2_ch = chunked_sb(whb2_sb[:], CW2)

        def pe_warm(n):
            # keep the PE p-state ramped through DMA-wait windows
            for _ in range(n):
                scr = psum.tile([128, R], F32, tag="bank", name="warm")
                nc.tensor.matmul(scr[:], xrT_sb[:, 0:128], xrT_sb[:, 0:R],
                                 start=True, stop=True)

        # ---- phase 0: weight loads + constants ---- #
        xrT_dst = chunked_sb(xrT_sb[:], R)
        xrT_src = chunked(xrT, R)
        nc.sync.dma_start(xrT_dst[:, :, 0:256], xrT_src[:, :, 0:256])
        load(wa_sb, wa, 2 * H)
        load(W_all_sb, W_all, DALL)
        nc.sync.dma_start(xrT_dst[:, :, 256:R], xrT_src[:, :, 256:R])
        load(W_out_sb, W_out, D)
        load(wa2_sb, wa2, 2)
        nc.vector.memset(ones_sb[:], 1.0)
        nc.vector.memset(ones16_sb[:], 1.0)
        nc.vector.memset(neg1_sb[:], -1.0)
        masks.make_identity(nc, ident_sb[:])

        # ---- phase A: own-row Wh/f/g + G-row, striped allgather ---- #
        pfr_t = ppool.tile([16, R], F32, tag="pout", name="pfr")
        pfr = pfr_t[:]
        gx16 = res.tile([16, R], F16, tag="gx16")
        fgb_d = dram.tile([16, R], F16, tag="fgb")

        def pfr_emit():
            for kc in range(4):
                nc.tensor.matmul(
                    pfr, wa_sb[:, kc * 2 * H:(kc + 1) * 2 * H],
                    xrT_sb[:, kc * R:(kc + 1) * R],
                    start=(kc == 0), stop=(kc == 3),
                )
            nc.scalar.activation(gx16[:], pfr, AF.Exp, scale=1.0 - ALPHA)
            nc.gpsimd.dma_start(fgb_d[:], gx16[:])

        gt1s = [res.tile([128, 2 * CW1], F16, tag=f"gt1{s}", name=f"gt1{s}")
                for s in range(2)]
        for s in range(2):
            nc.gpsimd.memset(gt1s[s][:], 1.0)   # bakes the ones columns

        def own_block(ib):
            gt1 = gt1s[ib // 2][:, (ib % 2) * CW1:(ib % 2 + 1) * CW1]
            pw = psum.tile([128, DALL], F32, tag="bank")
            pf = psum.tile([128, 2 * H], F32, tag="bank")
            for kc in range(4):
                lhsT = xrT_sb[:, kc * R + ib * 128: kc * R + (ib + 1) * 128]
                nc.tensor.matmul(
                    pw[:], lhsT, W_all_sb[:, kc * DALL:(kc + 1) * DALL],
                    start=(kc == 0), stop=(kc == 3))
                nc.tensor.matmul(
                    pf[:], lhsT, wa_sb[:, kc * 2 * H:(kc + 1) * 2 * H],
                    start=(kc == 0), stop=(kc == 3))
            dst = gt1[:, 0:W1S].rearrange(
                "p (h x) -> p h x", x=AUG1)[:, :, 0:D]
            nc.scalar.activation(
                dst, pw.rearrange("p (h x) -> p h x", x=D), AF.Copy)
            nc.vector.tensor_copy(gt1[:, W1S:CW1], pf[:])

        cc_space = {} if emulate_collective else {"addr_space": "Shared"}
        RH = R // 2
        cc1_in = [dram.tile([RH, CW1], F16, tag=f"cc1_in{s}", name=f"cc1_in{s}") for s in range(2)]
        cc1_out = [dram.tile([NCORES * RH, CW1], F16, tag=f"cc1_out{s}",
                             name=f"cc1_out{s}", **cc_space) for s in range(2)]

        def gather_in(cc_in, src_sb_ch):
            nc.sync.dma_start(
                cc_in[:].rearrange("(c p) w -> p c w", p=128),
                src_sb_ch)

        def gather_piece(cc_in, cc_out, nchunks, c2):
            """Deliver cores [c2, c2+2)'s rows of the allgather output."""
            if emulate_collective:
                w = cc_in.shape[1]
                dst = cc_out[:].rearrange("(c q) w -> c q w", c=NCORES)
                nc.sync.dma_start(
                    dst[c2:c2 + 2],
                    cc_in[:].unsqueeze(0).broadcast_to(
                        [2, nchunks * 128, w]))
            elif c2 == 0:
                nc.gpsimd.collective_compute(
                    "AllGather", mybir.AluOpType.bypass,
                    replica_groups=[list(range(NCORES))],
                    ins=[cc_in.opt()], outs=[cc_out.opt()],
                )

        def gather(cc_in, cc_out, src_sb_ch, nchunks):
            gather_in(cc_in, src_sb_ch)
            for c2 in range(0, NCORES, 2):
                gather_piece(cc_in, cc_out, nchunks, c2)

        def land_stripe(s, cc_out, ch_ap, q0=0, q1=NCORES):
            # DMA APs are limited to 3 dims: one DMA per chunk-of-pair t
            src = cc_out[:].rearrange("(co t p) w -> p co t w", t=2, p=128)
            dst = ch_ap.rearrange("p (co fo) w -> p co fo w", fo=4)
            for t in range(2):
                nc.sync.dma_start(
                    dst[:, q0:q1, 2 * s + t, :],
                    src[:, q0:q1, t, :])

        def adj_stripe(s, q0=0, q1=NCORES, piece=2):
            src = adjB.ap().rearrange("(co fo p) w -> p co fo w", fo=4, p=128)
            dst = chunked_sb(adjB_sb[:], R).rearrange(
                "p (co fo) w -> p co fo w", fo=4)
            for t in range(2):
                for q in range(q0, q1, piece):
                    nc.sync.dma_start(
                        dst[:, q:q + piece, 2 * s + t, :],
                        src[:, q:q + piece, 2 * s + t, :])

        def l1_etiles(s, q0=0, q1=NCORES, step=NCORES):
            wview = whbig_sb[:].rearrange("p (co fo w) -> p co fo w", fo=4, w=CW1)
            step = min(step, q1 - q0)
            for q in range(q0, q1, step):
                for t in range(2):
                    gc = wview[:, q:q + step, 2 * s + t, W1S:CW1].rearrange(
                        "p co (h two) -> p co h two", two=2)[:, :, :, 1:2]
                    for e_sb, sc in ((eg_sb, 1.0), (e02_sb, ALPHA),
                                     (e08_sb, 1.0 - ALPHA)):
                        dst = e_sb[:].rearrange(
                            "p (co fo h) -> p co fo h", fo=4, h=H)[
                            :, q:q + step, 2 * s + t, :].unsqueeze(3)
                        nc.scalar.activation(dst, gc, AF.Exp, scale=sc)
                    esrc = e02_sb[:].rearrange(
                        "p (co fo h) -> p co fo h", fo=4, h=H)[
                        :, q:q + step, 2 * s + t, :]
                    edst = e02h_sb[:].rearrange(
                        "p (co fo h) -> p co fo h", fo=4, h=H)[
                        :, q:q + step, 2 * s + t, :]
                    nc.vector.tensor_copy(edst, esrc)

        def l1_whbt(stripe, only_g=None):
            for g in S_GROUPS:
                if (g // 4) != stripe or (only_g is not None and g != only_g):
                    continue
                for jc in GROUPS[g]:
                    k = S_CHUNKS[jc]
                    src = whbig_ch[:, jc, 0:W1S].rearrange(
                        "p (h x) -> p h x", x=AUG1)
                    fb = e02h_sb[:, jc * H:(jc + 1) * H].unsqueeze(2) \
                        .broadcast_to([128, H, AUG1])
                    nc.gpsimd.tensor_tensor(
                        whbt_sb[:, k * W1S:(k + 1) * W1S].rearrange(
                            "p (h x) -> p h x", x=AUG1),
                        src, fb, AT.mult)

        # emission order = DMA queue order: stripe A lands in co-pair
        # pieces (gather -> land -> exps -> whbt), each unlocking one group,
        # before stripe B so compute starts as early as possible.
        own_block(0)
        own_block(1)
        pe_warm(12)
        adj_stripe(0)
        gather_in(cc1_in[0], chunked_sb(gt1s[0][:], CW1))
        for c2 in range(4):
            gather_piece(cc1_in[0], cc1_out[0], 2, 2 * c2)
            land_stripe(0, cc1_out[0], whbig_ch, q0=2 * c2, q1=2 * c2 + 2)
            l1_etiles(0, q0=2 * c2, q1=2 * c2 + 2)
            if c2 in S_GROUPS:
                l1_whbt(0, only_g=c2)
        own_block(2)
        own_block(3)
        pfr_emit()
        gather(cc1_in[1], cc1_out[1], chunked_sb(gt1s[1][:], CW1), 2)
        land_stripe(1, cc1_out[1], whbig_ch)
        l1_etiles(1)
        l1_whbt(1)

        # ---- attention unit ---- #
        def unit_start(f_row_dram):
            pout = ppool.tile([AUG1, R], F32, tag="pout")
            frep = tpool.tile([128, R], F16, tag="frep")
            nc.sync.dma_start(frep[:], f_row_dram.broadcast_to([128, R]))
            return pout, frep

        def unit_group(unit, pout, frep, g, mm, lhsT_of, lhsTs_of,
                       eg_of, e02_of, e08_of):
            on_act = g in S_GROUPS
            chunks = GROUPS[g]
            u = work.tile([128, G * R], F16, tag="u")
            for c, jc in enumerate(chunks):
                if on_act:
                    nc.scalar.activation(
                        u[:, c * R:(c + 1) * R], frep[:],
                        AF.Prelu, bias=neg1_sb[:], scale=e08_of(jc),
                        alpha=0.0)
                else:
                    nc.vector.tensor_scalar(
                        u[:, c * R:(c + 1) * R], frep[:],
                        eg_of(jc), e02_of(jc), AT.mult, AT.max)
            pm = work2.tile([128, G * R], F16, tag="pm")
            eng = nc.gpsimd if _mask_on_pool(unit, g) else nc.vector
            for r in range(2):
                c0 = chunks[2 * r]
                eng.tensor_tensor(
                    pm[:, 2 * r * R:(2 * r + 2) * R],
                    u[:, 2 * r * R:(2 * r + 2) * R],
                    adjB_sb[:, c0 * R:(c0 + 2) * R], AT.mult)
            for c, jc in enumerate(chunks):
                lhsT = lhsTs_of(S_CHUNKS[jc]) if on_act else lhsT_of(jc)
                nc.tensor.matmul(
                    pout[:], lhsT, pm[:, c * R:(c + 1) * R],
                    start=(mm[0] == 0), stop=(mm[0] == mm[1] - 1))
                mm[0] += 1
            if on_act:
                for jc in chunks:
                    nc.tensor.matmul(
                        pout[:], lhsTs_of(S_CHUNKS[jc]),
                        adjB_sb[:, jc * R:(jc + 1) * R],
                        start=(mm[0] == 0), stop=(mm[0] == mm[1] - 1))
                    mm[0] += 1

        def epilogue(pout, dst_ap, dst_f32, tail=False):
            """dst = elu(att_out / rowsum) written to dst_ap ([64, R])."""
            dt = F32 if dst_f32 else F16
            eng = nc.vector if tail else nc.gpsimd
            recip = rpool.tile([1, R], F32, tag="recip")
            nc.vector.reciprocal(recip[:], pout[D:D + 1, :])
            pr = psum.tile([D, R], F32, tag="bank")
            nc.tensor.matmul(pr[:], ones_sb[0:1, 0:D], recip[:])
            rsb = small.tile([D, R], F32, tag="ep")
            nc.scalar.activation(rsb[:], pr[:], AF.Copy)
            hl = small.tile([D, R], dt, tag="ep")
            nc.vector.tensor_tensor(hl[:], pout[0:D, :], rsb[:], AT.mult)
            # elu(x) = max(x,0) + min(exp(x),1) - 1   (exp monotone)
            q = small.tile([D, R], dt, tag="ep")
            nc.scalar.activation(q[:], hl[:], AF.Exp)
            t1 = small.tile([D, R], dt, tag="ep")
            nc.vector.tensor_scalar(t1[:], q[:], 1.0, -1.0, AT.min, AT.add)
            t2 = small.tile([D, R], dt, tag="ep")
            eng.tensor_scalar(t2[:], hl[:], 0.0, None, AT.max)
            eng.tensor_tensor(dst_ap, t1[:], t2[:], AT.add)

        # ---- phase C: layer-1 heads, pairs w/ deferred epilogues ---- #
        # Layer-2 prep partials accumulate in SBUF (a PSUM accumulator held
        # open across the whole layer-1 phase corrupts on real HW).
        MM_TOTAL = JC + NS * G
        pw2acc = res.tile([128, 4 * CW2], F32, tag="pw2acc")
        pfg2 = res.tile([2, R], F32, tag="pfg2")

        def l2_accum(kc):
            pt2 = psum.tile([128, 4 * CW2], F32, tag="bank", name="pt2")
            for ib in range(4):
                lhsT = hcatT_sb[:, kc * R + ib * 128: kc * R + (ib + 1) * 128]
                nc.tensor.matmul(
                    pt2[:, ib * CW2: ib * CW2 + D],
                    lhsT, W_out_sb[:, kc * D:(kc + 1) * D],
                    start=True, stop=True)
                nc.tensor.matmul(
                    pt2[:, ib * CW2 + D: ib * CW2 + D + 2],
                    lhsT, wa2_sb[:, kc * 2:(kc + 1) * 2],
                    start=True, stop=True)
            ptf = psum.tile([2, R], F32, tag="bank", name="ptf")
            nc.tensor.matmul(ptf[:], wa2_sb[:, kc * 2:(kc + 1) * 2],
                             hcatT_sb[:, kc * R:(kc + 1) * R],
                             start=True, stop=True)
            if kc == 0:
                nc.vector.tensor_copy(pw2acc[:], pt2[:])
                nc.vector.tensor_copy(pfg2[:], ptf[:])
            else:
                nc.vector.tensor_tensor(pw2acc[:], pw2acc[:], pt2[:], AT.add)
                nc.vector.tensor_tensor(pfg2[:], pfg2[:], ptf[:], AT.add)

        def l1_args(h):
            return (
                lambda jc, h=h: whbig_ch[:, jc, h * AUG1:(h + 1) * AUG1],
                lambda k, h=h: whbt_sb[:, k * W1S + h * AUG1:
                                       k * W1S + (h + 1) * AUG1],
                lambda jc, h=h: eg_sb[:, jc * H + h: jc * H + h + 1],
                lambda jc, h=h: e02_sb[:, jc * H + h: jc * H + h + 1],
                lambda jc, h=h: e08_sb[:, jc * H + h: jc * H + h + 1],
            )

        prev_pair = None
        started = {}

        def ensure_started(hp):
            if hp not in started and hp < H:
                started[hp] = [
                    [h] + list(unit_start(fgb_d[2 * h:2 * h + 1, :]))
                    + [[0, MM_TOTAL], l1_args(h)]
                    for h in (hp, hp + 1)
                ]
            return started.get(hp)

        for hp in range(0, H, 2):
            pair = ensure_started(hp)
            for gi in range(NG):
                for pi, (h, pout, frep, mm, args) in enumerate(pair):
                    # stagger the pair by one group so one unit is in an
                    # ACT-built group while the other is in a DVE-built one;
                    # pair 0's second unit wraps within each stripe so both
                    # stay on stripe A while stripe B is still landing
                    if hp == 0 and pi == 1:
                        g = 4 * (gi // 4) + (gi + 1) % 4
                    else:
                        g = (gi + pi) % NG
                    unit_group(h, pout, frep, g, mm, *args)
                if gi == 0 and prev_pair is None and hp == 0:
                    adj_stripe(1)
                # spread the previous pair's epilogues and the layer-2
                # partial accumulation across three group slots so their
                # cross-engine chains don't block the in-order queues
                if prev_pair is not None and gi == 0:
                    for (h, pout, frep, mm, args) in prev_pair:
                        kc, po = h // 2, (h % 2) * D
                        epilogue(pout,
                                 hcatT_sb[po:po + D, kc * R:(kc + 1) * R],
                                 dst_f32=False)
                if prev_pair is not None and gi == 1:
                    l2_accum(prev_pair[0][0] // 2)
                if gi == NG - 2:
                    ensure_started(hp + 2)
            prev_pair = pair
        for (h, pout, frep, mm, args) in prev_pair:
            kc, po = h // 2, (h % 2) * D
            epilogue(pout, hcatT_sb[po:po + D, kc * R:(kc + 1) * R],
                     dst_f32=False, tail=True)
        l2_accum(3)

        # ---- phase D: layer-2 gather (striped) ---- #
        gt2s = [res.tile([128, 2 * CW2], F16, tag=f"gt2{s}", name=f"gt2{s}")
                for s in range(2)]
        for s in range(2):
            nc.vector.memset(gt2s[s][:], 1.0)
        for ib in range(4):
            gt2 = gt2s[ib // 2][:, (ib % 2) * CW2:(ib % 2 + 1) * CW2]
            nc.vector.tensor_copy(
                gt2[:, 0:D], pw2acc[:, ib * CW2: ib * CW2 + D])
            nc.vector.tensor_copy(
                gt2[:, D + 1:D + 2],
                pw2acc[:, ib * CW2 + D + 1: ib * CW2 + D + 2])

        # broadcast G2row across partitions via PE instead of a DRAM bounce
        g2row = res.tile([1, R], F16, tag="g2row")
        nc.scalar.activation(g2row[:], pfg2[0:1, :], AF.Exp, scale=1.0 - ALPHA)
        frep2 = tpool.tile([128, R], F16, tag="frep")
        pb2 = psum.tile([128, R], F32, tag="bank")
        nc.tensor.matmul(pb2[:], ones16_sb[0:1, 0:128], g2row[:],
                         start=True, stop=True)
        nc.vector.tensor_copy(frep2[:], pb2[:])

        cc2_in = [dram.tile([RH, CW2], F16, tag=f"cc2_in{s}", name=f"cc2_in{s}") for s in range(2)]
        cc2_out = [dram.tile([NCORES * RH, CW2], F16, tag=f"cc2_out{s}",
                             name=f"cc2_out{s}", **cc_space) for s in range(2)]

        def l2_etiles(s):
            wview = whb2_sb[:].rearrange("p (co fo w) -> p co fo w", fo=4, w=CW2)
            for t in range(2):
                gc = wview[:, :, 2 * s + t, D + 1:D + 2]
                for e_sb, sc in ((eg2_sb, 1.0), (e022_sb, ALPHA),
                                 (e082_sb, 1.0 - ALPHA)):
                    dst = e_sb[:].rearrange(
                        "p (co fo) -> p co fo", fo=4)[:, :, 2 * s + t] \
                        .unsqueeze(2)
                    nc.scalar.activation(dst, gc, AF.Exp, scale=sc)

        def l2_whbt(stripe):
            for g in S_GROUPS:
                if (g // 4) != stripe:
                    continue
                for jc in GROUPS[g]:
                    k = S_CHUNKS[jc]
                    nc.vector.tensor_scalar(
                        whb2t_sb[:, k * AUG1:(k + 1) * AUG1],
                        whb2_ch[:, jc, 0:AUG1],
                        e022_sb[:, jc:jc + 1], None, AT.mult)

        # ---- phase E: layer 2, stripe-pipelined ---- #
        pout2 = ppool.tile([AUG1, R], F32, tag="pout")
        mm2 = [0, MM_TOTAL]
        args2 = (
            lambda jc: whb2_ch[:, jc, 0:AUG1],
            lambda k: whb2t_sb[:, k * AUG1:(k + 1) * AUG1],
            lambda jc: eg2_sb[:, jc:jc + 1],
            lambda jc: e022_sb[:, jc:jc + 1],
            lambda jc: e082_sb[:, jc:jc + 1],
        )
        for s in range(2):
            gather(cc2_in[s], cc2_out[s], chunked_sb(gt2s[s][:], CW2), 2)
            land_stripe(s, cc2_out[s], whb2_ch)
            l2_etiles(s)
            l2_whbt(s)

        def pe_warm2(n):
            # gt2-gated fillers: run during the layer-2 gather wait
            for _ in range(n):
                scr = psum.tile([128, 2 * CW2], F32, tag="bank", name="warm2")
                nc.tensor.matmul(scr[:], gt2s[0][:, 0:128], gt2s[0][:],
                                 start=True, stop=True)

        pe_warm2(30)
        for g in range(NG):
            unit_group(H, pout2, frep2, g, mm2, *args2)
        # final epilogue, block-pipelined with the transposes and out DMA
        recip2 = rpool.tile([1, R], F32, tag="recip")
        nc.vector.reciprocal(recip2[:], pout2[D:D + 1, :])
        pr2 = psum.tile([D, R], F32, tag="bank")
        nc.tensor.matmul(pr2[:], ones_sb[0:1, 0:D], recip2[:])
        out_ch = out.ap().rearrange("(c p) w -> p c w", p=128)
        for ib in range(4):
            cs = slice(ib * 128, (ib + 1) * 128)
            rsb = small.tile([D, 128], F32, tag="ep")
            nc.scalar.activation(rsb[:], pr2[:, cs], AF.Copy)
            hl = small.tile([D, 128], F32, tag="ep")
            nc.vector.tensor_tensor(hl[:], pout2[0:D, cs], rsb[:], AT.mult)
            q = small.tile([D, 128], F32, tag="ep")
            nc.scalar.activation(q[:], hl[:], AF.Exp)
            t1 = small.tile([D, 128], F32, tag="ep")
            nc.vector.tensor_scalar(t1[:], q[:], 1.0, -1.0, AT.min, AT.add)
            t2 = small.tile([D, 128], F32, tag="ep")
            nc.gpsimd.tensor_scalar(t2[:], hl[:], 0.0, None, AT.max)
            r2 = small.tile([D, 128], F32, tag="ep2")
            nc.gpsimd.tensor_tensor(r2[:], t1[:], t2[:], AT.add)
            pt = psum.tile([128, D], F32, tag="bank")
            nc.tensor.transpose(pt[:], r2[:], ident_sb[:])
            nc.vector.tensor_copy(out_sb[:, ib * D:(ib + 1) * D], pt[:])
            nc.sync.dma_start(out_ch[:, ib, :],
                              chunked_sb(out_sb[:], D)[:, ib, :])


# --------------------------------------------------------------------------- #
# host side
# --------------------------------------------------------------------------- #

def _pack_inputs(x, adj, W_heads, a_src, a_dst, W_out, a_src_out, a_dst_out):
    """Shard + repack the full inputs into the 8 per-core input maps."""
    x = np.asarray(x, np.float32)
    adj = np.asarray(adj)
    W_heads = np.asarray(W_heads, np.float32)
    a_src = np.asarray(a_src, np.float32)
    a_dst = np.asarray(a_dst, np.float32)
    W_out_np = np.asarray(W_out, np.float32)
    a_src_out = np.asarray(a_src_out, np.float32)
    a_dst_out = np.asarray(a_dst_out, np.float32)

    f16 = NPF16
    W_all = np.ascontiguousarray(
        W_heads.transpose(1, 0, 2).reshape(K, DALL)).astype(f16)     # [K, H*D]
    wa_cols = []
    for h in range(H):
        wa_cols.append(W_heads[h] @ a_src[h])
        wa_cols.append(W_heads[h] @ a_dst[h])
    wa = np.stack(wa_cols, axis=1).astype(f16)                       # [K, 16]
    W_out_p = W_out_np.astype(f16)                                   # [DALL, D]
    wa2 = np.stack([W_out_np @ a_src_out, W_out_np @ a_dst_out],
                   axis=1).astype(f16)                               # [DALL, 2]

    in_maps = []
    for c in range(NCORES):
        rows = slice(c * R, (c + 1) * R)
        adj_rows = (adj[rows, :] > 0).astype(np.float32)             # [R, N]
        adjB = np.ascontiguousarray(adj_rows.T).astype(f16)          # [N, R] 0/1
        in_maps.append({
            "xrT": np.ascontiguousarray(x[rows].T).astype(f16),
            "adjB": adjB,
            "W_all": W_all,
            "wa": wa,
            "W_out": W_out_p,
            "wa2": wa2,
        })
    return in_maps


def kernel(**inputs) -> np.ndarray:
    if "nc" not in _CACHE:
        _CACHE["nc"] = _build(emulate_collective=False)
    nc = _CACHE["nc"]
    in_maps = _pack_inputs(**inputs)
    res = run_bass_kernel_spmd(nc, in_maps, core_ids=list(range(NCORES)))
    return np.concatenate([res.results[c]["out"] for c in range(NCORES)], axis=0)
